# revision 1
# baseline (speedup 1.0000x reference)
"""TRN2 Bass kernel for nn_HCSMoEQwen3MoeSparseMoeBlock (8-core expert-parallel).

Sharding: core g owns group g's dominant expert and processes ALL tokens;
router replicated (each core computes only its group's combined weight
w_g[t]); host sums the 8 partial outputs w_g[t] * y_g[t, :].

Single software-pipelined loop over 128-token chunks; float32r matmuls
(full PE rate, ~2e-4 rel err); router logits in exact fp32 (separate
F32-typed tiles — the PE precision mode follows the backing tensor dtype):
  router: logitsT = gwT.T-stationary @ x-chunk (fp32) -> PE transpose
  M1 b-major: h_b = xT_c.T @ gu_b, 16 same-bank MMs per 512-col block
              (host-interleaved [256 gate|256 up]) -> silu+mult drains bank
  actT = PE-transpose(act);  y = actT.T @ dnT;  top-8 chain on DVE
  (pinned after casts);  out = w*y -> DRAM
"""
import numpy as np

import concourse.bass as bass
import concourse.mybir as mybir
import concourse.tile as tile
from concourse import bacc
from concourse.bass_utils import run_bass_kernel_spmd
from concourse.masks import make_identity

T = 2048
H = 2048
I2 = 1536
I = 768
E = 32
G = 8
TOP_K = 8
KO = H // 128
JO = I // 128
TCH = 128
NCHUNK = T // TCH
HB = 512
NEG_BIG = -1.0e9

F32 = mybir.dt.float32
F32R = mybir.dt.float32r
U8 = mybir.dt.uint8
AX = mybir.AxisListType.X
OP = mybir.AluOpType
ACTF = mybir.ActivationFunctionType

_CACHED_NC = None


def _build():
    global _CACHED_NC
    if _CACHED_NC is not None:
        return _CACHED_NC
    nc = bacc.Bacc("TRN2", target_bir_lowering=False, debug=False, num_devices=G)

    xT_d = nc.dram_tensor("xT", [H, T], F32R, kind="ExternalInput")
    gu_d = nc.dram_tensor("gu", [H, I2], F32R, kind="ExternalInput")
    gw_d = nc.dram_tensor("gw", [H, E], F32, kind="ExternalInput")
    dnT_d = nc.dram_tensor("dnT", [I, H], F32R, kind="ExternalInput")
    mgb_d = nc.dram_tensor("mgb", [128, E], F32, kind="ExternalInput")
    y_d = nc.dram_tensor("y", [T, H], F32, kind="ExternalOutput")

    xT_ap = xT_d.ap().rearrange("(ko p) t -> p ko t", p=128)
    xT_ap32 = xT_d.ap().bitcast(F32).rearrange("(ko p) t -> p ko t", p=128)
    gu_ap = gu_d.ap().rearrange("(ko p) o -> p ko o", p=128)
    gw_ap = gw_d.ap().rearrange("(ko p) e -> p ko e", p=128)
    dnT_ap = dnT_d.ap().rearrange("(jo p) h -> p jo h", p=128)

    with tile.TileContext(nc) as tc:
        with (
            tc.tile_pool(name="const", bufs=1) as cpool,
            tc.tile_pool(name="weights", bufs=1) as wpool,
            tc.tile_pool(name="xin", bufs=2) as xpool,
            tc.tile_pool(name="xrin", bufs=2) as xrpool,
            tc.tile_pool(name="acts", bufs=1) as apool,
            tc.tile_pool(name="router", bufs=2) as rpool,
            tc.tile_pool(name="yout", bufs=2) as ypool,
            tc.tile_pool(name="plg", bufs=1, space="PSUM") as plg,
            tc.tile_pool(name="ph", bufs=3, space="PSUM") as pph,
            tc.tile_pool(name="ps", bufs=1, space="PSUM") as pps,
            tc.tile_pool(name="py", bufs=2, space="PSUM") as ppy,
        ):
            identity = cpool.tile([128, 128], F32, tag="identity")
            make_identity(nc, identity)
            negbig = cpool.tile([128, E], F32, tag="negbig")
            nc.vector.memset(negbig, NEG_BIG)
            mgb_sb = cpool.tile([128, E], F32, tag="mgb")
            nc.sync.dma_start(mgb_sb[:], mgb_d.ap())
            gw_sb = cpool.tile([128, KO, E], F32, tag="gw")
            nc.sync.dma_start(gw_sb[:], gw_ap)

            gu_sb = wpool.tile([128, KO, I2], F32R, tag="gu")
            dn_sb = wpool.tile([128, JO, H], F32R, tag="dn")

            xtiles = {}
            xrtiles = {}

            def load_chunk(ci):
                t = xpool.tile([128, KO, TCH], F32R, tag="xT_c",
                               name=f"xT_c{ci}")
                nc.sync.dma_start(t[:], xT_ap[:, :, ci * TCH:(ci + 1) * TCH])
                xtiles[ci] = t

            def load_xr(ci):
                t = xrpool.tile([128, KO, TCH], F32, tag="xr",
                                name=f"xr{ci}")
                nc.sync.dma_start(t[:], xT_ap32[:, :, ci * TCH:(ci + 1) * TCH])
                xrtiles[ci] = t

            load_xr(0)
            load_chunk(0)
            nc.sync.dma_start(gu_sb[:, :, 0:HB], gu_ap[:, :, 0:HB])

            for tci in range(NCHUNK):
                tsl = slice(tci * TCH, (tci + 1) * TCH)
                if tci + 1 < NCHUNK:
                    load_xr(tci + 1)
                    load_chunk(tci + 1)
                xT_c = xtiles.pop(tci)
                xr_c = xrtiles.pop(tci)

                # ---- router logits (exact fp32), gw stationary ----
                lg_ps = plg.tile([E, TCH], F32, tag="lg_ps")
                for k in range(KO):
                    nc.tensor.matmul(
                        lg_ps[:], gw_sb[:, k], xr_c[:, k],
                        start=(k == 0), stop=(k == KO - 1),
                    )
                lgT_sb = rpool.tile([E, TCH], F32, tag="lgT_sb")
                nc.vector.tensor_copy(lgT_sb[:], lg_ps[:])
                s_ps = pps.tile([128, JO + 1, TCH], F32, tag="s_ps")
                nc.tensor.transpose(s_ps[:, JO, :E], lgT_sb[:],
                                    identity[:E, :E])
                logits = rpool.tile([128, E], F32, tag="logits")
                nc.vector.tensor_copy(logits[:], s_ps[:, JO, :E])

                if tci == 0:
                    nc.sync.dma_start(gu_sb[:, :, HB:2 * HB],
                                      gu_ap[:, :, HB:2 * HB])

                # ---- M1, b-major: one PSUM bank at a time ----
                act_sb = apool.tile([128, I], F32, tag="act")
                for b in range(3):
                    h_ps = pph.tile([128, HB], F32, tag="h_ps",
                                    name=f"h{tci}_{b}")
                    for k in range(KO):
                        nc.tensor.matmul(
                            h_ps[:], xT_c[:, k],
                            gu_sb[:, k, b * HB:(b + 1) * HB],
                            start=(k == 0), stop=(k == KO - 1),
                        )
                    if tci == 0 and b == 0:
                        nc.sync.dma_start(gu_sb[:, :, 2 * HB:I2],
                                          gu_ap[:, :, 2 * HB:I2])
                    if tci == 0 and b == 1:
                        for j in range(JO):
                            nc.sync.dma_start(dn_sb[:, j], dnT_ap[:, j])
                    # host interleave: h_b = [256 gate | 256 up]
                    silu_sb = apool.tile([128, 256], F32, tag="silu")
                    nc.scalar.activation(silu_sb[:], h_ps[:, :256],
                                         ACTF.Silu)
                    nc.vector.tensor_tensor(
                        act_sb[:, 256 * b:256 * (b + 1)], silu_sb[:],
                        h_ps[:, 256:], OP.mult,
                    )

                # ---- transpose act -> actT ----
                actT_sb = apool.tile([128, JO, TCH], F32R, tag="actT")
                for j in range(JO):
                    nc.tensor.transpose(
                        s_ps[:, j], act_sb[:, j * 128:(j + 1) * 128],
                        identity,
                    )
                    nc.vector.tensor_copy(actT_sb[:, j], s_ps[:, j])

                # ---- M2 ----
                y_pss = []
                for hb in range(H // HB):
                    y_ps = ppy.tile([128, HB], F32, tag="y_ps",
                                    name=f"y_ps{tci}_{hb}")
                    for j in range(JO):
                        nc.tensor.matmul(
                            y_ps[:], actT_sb[:, j],
                            dn_sb[:, j, hb * HB:(hb + 1) * HB],
                            start=(j == 0), stop=(j == JO - 1),
                        )
                    y_pss.append(y_ps)

                # ---- top-8 router chain (DVE) ----
                # Pin the chain after the casts so it can't hog DVE while the
                # next chunk's SwiGLU needs the h banks released. The first
                # chunk has an idle DVE (DMA-bound head) and the last has no
                # successor to protect, so let those chains run early.
                cur = rpool.tile([128, E], F32, tag="cur")
                if 0 < tci < NCHUNK - 1:
                    dep = rpool.tile([128, E], F32, tag="dep")
                    nc.vector.tensor_scalar(
                        dep[:], actT_sb[:, JO - 1, :E].bitcast(F32), 0.0,
                        None, OP.mult)
                    nc.vector.tensor_tensor(cur[:], logits[:], dep[:], OP.add)
                else:
                    nc.vector.tensor_copy(cur[:], logits[:])
                msk = rpool.tile([128, E], U8, tag="msk")
                m1 = rpool.tile([128, 1], F32, tag="m1")
                mk = rpool.tile([128, 1], F32, tag="mk")
                for it in range(TOP_K - 1):
                    tgt = m1 if it == 0 else mk
                    nc.vector.reduce_max(tgt[:], cur[:], axis=AX)
                    nc.vector.tensor_scalar(msk[:], cur[:], tgt[:],
                                            None, OP.is_ge)
                    nc.vector.copy_predicated(cur[:], msk[:], negbig[:])
                m8 = rpool.tile([128, 1], F32, tag="m8")
                nc.vector.reduce_max(m8[:], cur[:], axis=AX)

                nm1 = rpool.tile([128, 1], F32, tag="nm1")
                nc.vector.tensor_scalar(nm1[:], m1[:], -1.0, None, OP.mult)
                mask8 = rpool.tile([128, E], F32, tag="mask8")
                nc.vector.tensor_scalar(mask8[:], logits[:], m8[:],
                                        None, OP.is_ge)
                ew = rpool.tile([128, E], F32, tag="ew")
                nc.scalar.activation(ew[:], logits[:], ACTF.Exp, bias=nm1[:])
                nc.vector.tensor_tensor(ew[:], ew[:], mask8[:], OP.mult)
                s8 = rpool.tile([128, 1], F32, tag="s8")
                nc.vector.reduce_sum(s8[:], ew[:], axis=AX)
                nc.vector.tensor_tensor(ew[:], ew[:], mgb_sb[:], OP.mult)
                num = rpool.tile([128, 1], F32, tag="num")
                nc.vector.reduce_sum(num[:], ew[:], axis=AX)
                rs = rpool.tile([128, 1], F32, tag="rs")
                nc.vector.reciprocal(rs[:], s8[:])
                w_t = rpool.tile([128, 1], F32, tag="w_t")
                nc.vector.tensor_tensor(w_t[:], num[:], rs[:], OP.mult)

                # ---- scale + store ----
                for hb in range(H // HB):
                    y_sb = ypool.tile([128, HB], F32, tag="y_sb")
                    nc.vector.tensor_scalar(
                        y_sb[:], y_pss[hb][:], w_t[:], None, OP.mult,
                    )
                    nc.sync.dma_start(
                        y_d.ap()[tsl, hb * HB:(hb + 1) * HB], y_sb[:],
                    )
    nc.compile()
    _CACHED_NC = nc
    return nc


_GATEUP_PERM = np.concatenate(
    [np.r_[256 * b:256 * b + 256, 768 + 256 * b:768 + 256 * b + 256]
     for b in range(3)]
)


def prepare_in_maps(hidden_states, gate_weight, gate_up_proj, down_proj,
                    merge_groups, dominant_experts):
    x = np.asarray(hidden_states, dtype=np.float32).reshape(T, H)
    xT = np.ascontiguousarray(x.T)
    gw = np.asarray(gate_weight, dtype=np.float32)
    gwT = np.ascontiguousarray(gw.T)
    mg = np.asarray(merge_groups).astype(np.int64)
    de = np.asarray(dominant_experts).astype(np.int64)
    gup = np.asarray(gate_up_proj, dtype=np.float32)
    dnp_ = np.asarray(down_proj, dtype=np.float32)

    in_maps = []
    for g in range(G):
        e = int(de[g])
        guT = np.ascontiguousarray(gup[e].T[:, _GATEUP_PERM])
        dnT = np.ascontiguousarray(dnp_[e].T)
        mgb = np.ascontiguousarray(
            np.broadcast_to((mg == g).astype(np.float32)[None, :], (128, E))
        )
        in_maps.append({"xT": xT, "gu": guT, "gw": gwT, "dnT": dnT,
                        "mgb": mgb})
    return in_maps


def kernel(hidden_states, gate_weight, gate_up_proj, down_proj,
           merge_groups, dominant_experts):
    in_maps = prepare_in_maps(hidden_states, gate_weight, gate_up_proj,
                              down_proj, merge_groups, dominant_experts)
    nc = _build()
    res = run_bass_kernel_spmd(nc, in_maps, core_ids=list(range(G)),
                               trace=False)
    out = np.zeros((T, H), dtype=np.float64)
    for r in res.results:
        out += r["y"].astype(np.float64)
    return out.astype(np.float32).reshape(1, T, H)



# revision 3
# speedup vs baseline: 2.0445x; 2.0445x over previous
"""TRN2 Bass kernel for nn_HCSMoEQwen3MoeSparseMoeBlock (8-core, v2).

Host computes the router (fp32 numpy, matches reference softmax/top-8
semantics) and the per-(token, group) combined weight w_tg.  Only
(token, group) pairs with w > 0 are computed on device (~64% density).

Schedule: active tokens of each group are packed into 128-token chunks;
chunks are grouped into single-group segments of 4 or 3 chunks such that
every core runs an identical [4, 4, 3]-chunk program (3 segments, 1408
token slots) — 16 four-segments + 8 three-segments total, found by a tiny
DP over group chunk counts.  Per-core inputs: gathered xT (bf16), one
gu/dn weight slot per segment (bf16, gate/up column-interleaved), and the
per-token weights.  Host scatter-adds the fp32 partial outputs.

Device per segment (Ts = 512|384 tokens):
  M1 transposed: hT[o-tile, t] = guT-tile.T-stationary @ xT (bf16, fp32
  PSUM), gate/up PSUM bank pair -> silu * up fused on Scalar+DVE ->
  actT [i, t] bf16 (no PE transposes, ap_size >= 384 full rate).
  M2: y[t, hb] = actT-subtile stationary @ dnT (bf16) -> scale by w -> DRAM.
"""
import numpy as np
import ml_dtypes

import concourse.bass as bass
import concourse.mybir as mybir
import concourse.tile as tile
from concourse import bacc
from concourse.bass_utils import run_bass_kernel_spmd

BF = ml_dtypes.bfloat16

T = 2048
H = 2048
I2 = 1536
I = 768
E = 32
G = 8
TOP_K = 8
KO = H // 128          # 16 k-tiles
JO = I // 128          # 6 i-tiles
NSEG = 3
SEGC = (4, 4, 3)       # chunks per segment (identical on every core)
SEGT = tuple(128 * c for c in SEGC)
CHOFF = (0, 4, 8)      # chunk index offset per segment
TOFF = (0, 512, 1024)  # token slot offset per segment
NCH = sum(SEGC)        # 11 chunks per core
NT = 128 * NCH         # 1408 token slots per core
NF_TOT = 16            # total 4-chunk segments across cores
NT_TOT = 8             # total 3-chunk segments across cores
HBW = 512

F32 = mybir.dt.float32
BF16 = mybir.dt.bfloat16
AX = mybir.AxisListType.X
OP = mybir.AluOpType
ACTF = mybir.ActivationFunctionType

_CACHED_NC = None

# gate j-tile at cols [256j, 256j+128), up j-tile at [256j+128, 256j+256)
_GU_PERM = np.concatenate(
    [np.r_[128 * j:128 * j + 128, I + 128 * j:I + 128 * j + 128]
     for j in range(JO)]
)


def _build():
    global _CACHED_NC
    if _CACHED_NC is not None:
        return _CACHED_NC
    nc = bacc.Bacc("TRN2", target_bir_lowering=False, debug=False,
                   num_devices=G)

    xT_d = nc.dram_tensor("xT", [H, NT], BF16, kind="ExternalInput")
    gu_d = nc.dram_tensor("gu", [NSEG * H, I2], BF16, kind="ExternalInput")
    dn_d = nc.dram_tensor("dn", [NSEG * I, H], BF16, kind="ExternalInput")
    w_d = nc.dram_tensor("w", [128, NCH], F32, kind="ExternalInput")
    y_d = nc.dram_tensor("y", [NT, H], F32, kind="ExternalOutput")

    xT_ap = xT_d.ap().rearrange("(ko p) t -> p ko t", p=128)
    gu_ap = gu_d.ap().rearrange("(s ko p) o -> p s ko o", p=128, ko=KO)
    dn_ap = dn_d.ap().rearrange("(s jo p) h -> p s jo h", p=128, jo=JO)

    with tile.TileContext(nc) as tc:
        with (
            tc.tile_pool(name="const", bufs=1) as cpool,
            tc.tile_pool(name="guw", bufs=2) as gupool,
            tc.tile_pool(name="dnw", bufs=1) as dnpool,
            tc.tile_pool(name="xin", bufs=2) as xpool,
            tc.tile_pool(name="acts", bufs=2) as apool,
            tc.tile_pool(name="silu", bufs=2) as spool,
            tc.tile_pool(name="yout", bufs=4) as ypool,
            tc.tile_pool(name="ph", bufs=2, space="PSUM") as pph,
            tc.tile_pool(name="py", bufs=4, space="PSUM") as ppy,
        ):
            w_sb = cpool.tile([128, NCH], F32, tag="w")
            nc.sync.dma_start(w_sb[:], w_d.ap())

            x_tiles = {}
            gu_tiles = {}
            dn_tiles = {}

            def load_x(s):
                t = xpool.tile([128, KO, 512], BF16, tag="x", name=f"x{s}")
                nc.sync.dma_start(t[:, :, :SEGT[s]],
                                  xT_ap[:, :, TOFF[s]:TOFF[s] + SEGT[s]])
                x_tiles[s] = t

            def load_gu(s, pieces):
                if s not in gu_tiles:
                    gu_tiles[s] = gupool.tile([128, KO, I2], BF16, tag="gu",
                                              name=f"gu{s}")
                t = gu_tiles[s]
                for pc in pieces:
                    nc.sync.dma_start(
                        t[:, :, 512 * pc:512 * (pc + 1)],
                        gu_ap[:, s, :, 512 * pc:512 * (pc + 1)],
                    )

            def load_dn(s):
                t = dnpool.tile([128, JO, H], BF16, tag="dn", name=f"dn{s}")
                nc.sync.dma_start(t[:], dn_ap[:, s])
                dn_tiles[s] = t

            load_x(0)
            load_gu(0, (0, 1, 2))
            load_dn(0)

            for s in range(NSEG):
                Ts = SEGT[s]
                xc = x_tiles.pop(s)
                guc = gu_tiles.pop(s)

                actT = apool.tile([128, JO, 512], BF16, tag="act",
                                  name=f"act{s}")
                for j in range(JO):
                    a_ps = pph.tile([128, 512], F32, tag="hg",
                                    name=f"hg{s}_{j}")
                    b_ps = pph.tile([128, 512], F32, tag="hu",
                                    name=f"hu{s}_{j}")
                    for k in range(KO):
                        nc.tensor.matmul(
                            a_ps[:, :Ts], guc[:, k, 256 * j:256 * j + 128],
                            xc[:, k, :Ts],
                            start=(k == 0), stop=(k == KO - 1),
                        )
                    for k in range(KO):
                        nc.tensor.matmul(
                            b_ps[:, :Ts],
                            guc[:, k, 256 * j + 128:256 * j + 256],
                            xc[:, k, :Ts],
                            start=(k == 0), stop=(k == KO - 1),
                        )
                    sl = spool.tile([128, 512], F32, tag="sl",
                                    name=f"sl{s}_{j}")
                    nc.scalar.activation(sl[:, :Ts], a_ps[:, :Ts], ACTF.Silu)
                    nc.vector.tensor_tensor(actT[:, j, :Ts], sl[:, :Ts],
                                            b_ps[:, :Ts], OP.mult)
                    # stagger next-segment prefetches behind early compute
                    if s + 1 < NSEG:
                        if j == 1:
                            load_x(s + 1)
                        elif j in (2, 3, 4):
                            load_gu(s + 1, (j - 2,))

                dnc = dn_tiles.pop(s)
                for sub in range(Ts // 128):
                    for hb in range(H // HBW):
                        y_ps = ppy.tile([128, HBW], F32, tag="y_ps",
                                        name=f"y{s}_{sub}_{hb}")
                        for j in range(JO):
                            nc.tensor.matmul(
                                y_ps[:],
                                actT[:, j, 128 * sub:128 * (sub + 1)],
                                dnc[:, j, HBW * hb:HBW * (hb + 1)],
                                start=(j == 0), stop=(j == JO - 1),
                            )
                        y_sb = ypool.tile([128, HBW], F32, tag="y_sb",
                                          name=f"ys{s}_{sub}_{hb}")
                        ci = CHOFF[s] + sub
                        nc.vector.tensor_scalar(y_sb[:], y_ps[:],
                                                w_sb[:, ci:ci + 1], None,
                                                OP.mult)
                        nc.sync.dma_start(
                            y_d.ap()[TOFF[s] + 128 * sub:
                                     TOFF[s] + 128 * (sub + 1),
                                     HBW * hb:HBW * (hb + 1)],
                            y_sb[:],
                        )
                if s + 1 < NSEG:
                    load_dn(s + 1)
    nc.compile()
    _CACHED_NC = nc
    return nc


def _route(x32, gw32, mg):
    """fp32 router matching reference: softmax, top-8, renorm, per-group sum."""
    logits = x32 @ gw32.T
    m = logits.max(axis=-1, keepdims=True)
    p = np.exp(logits - m)
    p /= p.sum(axis=-1, keepdims=True)
    idx = np.argsort(-p, axis=-1, kind="stable")[:, :TOP_K]
    val = np.take_along_axis(p, idx, axis=-1)
    val = val / val.sum(axis=-1, keepdims=True)
    sel = mg[idx]  # [T, K] group ids
    w_tg = np.zeros((x32.shape[0], G), dtype=np.float32)
    np.add.at(w_tg, (np.arange(x32.shape[0])[:, None], sel), val)
    return w_tg


def _solve_segments(counts):
    """Split per-group 128-chunk counts into exactly NF_TOT 4-chunk and
    NT_TOT 3-chunk single-group segments (padding as needed).
    Returns per-group (n_four, n_three)."""
    states = {(0, 0): (0, ())}
    for c in counts:
        new = {}
        for (F, T3), (pad, plan) in states.items():
            for t3 in range(0, NT_TOT - T3 + 1):
                rem = c - 3 * t3
                fmin = max(0, -(-rem // 4))
                for f in range(fmin, NF_TOT - F + 1):
                    p = 4 * f + 3 * t3 - c
                    key = (F + f, T3 + t3)
                    cand = (pad + p, plan + ((f, t3),))
                    if key not in new or cand[0] < new[key][0]:
                        new[key] = cand
        states = new
    if (NF_TOT, NT_TOT) not in states:
        raise RuntimeError(f"no [4,4,3] schedule for chunk counts {counts}")
    return states[(NF_TOT, NT_TOT)][1]


def _schedule(w_tg):
    """Build 16 four-segments + 8 three-segments of (group, token_slots)."""
    counts = []
    toks = []
    for g in range(G):
        t = np.nonzero(w_tg[:, g] > 0.0)[0]
        toks.append(t)
        counts.append(-(-len(t) // 128))
    plan = _solve_segments(counts)
    fours, threes = [], []
    for g, (f, t3) in enumerate(plan):
        cap = 128 * (4 * f + 3 * t3)
        slots = np.full(cap, -1, dtype=np.int64)
        slots[:len(toks[g])] = toks[g]
        off = 0
        for _ in range(f):
            fours.append((g, slots[off:off + 512]))
            off += 512
        for _ in range(t3):
            threes.append((g, slots[off:off + 384]))
            off += 384
    return fours, threes


def prepare_in_maps(hidden_states, gate_weight, gate_up_proj, down_proj,
                    merge_groups, dominant_experts):
    x32 = np.asarray(hidden_states, dtype=np.float32).reshape(T, H)
    gw32 = np.asarray(gate_weight, dtype=np.float32)
    mg = np.asarray(merge_groups).astype(np.int64)
    de = np.asarray(dominant_experts).astype(np.int64)
    gup = np.asarray(gate_up_proj, dtype=np.float32)
    dnp_ = np.asarray(down_proj, dtype=np.float32)

    w_tg = _route(x32, gw32, mg)
    fours, threes = _schedule(w_tg)

    # per-expert transformed weights (bf16), cached across cores
    guT_c, dnT_c = {}, {}
    for g in range(G):
        e = int(de[g])
        if e not in guT_c:
            guT_c[e] = np.ascontiguousarray(
                gup[e].T[:, _GU_PERM]).astype(BF)            # [H, 2I]
            dnT_c[e] = np.ascontiguousarray(dnp_[e].T).astype(BF)  # [I, H]

    x_bf = x32.astype(BF)
    in_maps = []
    slot_lists = []
    for c in range(G):
        segs = [fours[2 * c], fours[2 * c + 1], threes[c]]
        slots = np.concatenate([sg[1] for sg in segs])       # [NT], -1 pads
        slot_lists.append(slots)
        cl = np.where(slots < 0, 0, slots)
        w_slots = np.zeros(NT, dtype=np.float32)
        pos = 0
        for g, sl in segs:
            n = len(sl)
            valid = sl >= 0
            w_slots[pos:pos + n][valid] = w_tg[sl[valid], g]
            pos += n
        xT = np.ascontiguousarray(x_bf[cl].T)                # [H, NT]
        gu_dev = np.concatenate([guT_c[int(de[g])] for g, _ in segs], axis=0)
        dn_dev = np.concatenate([dnT_c[int(de[g])] for g, _ in segs], axis=0)
        w_dev = np.ascontiguousarray(w_slots.reshape(NCH, 128).T)
        in_maps.append({"xT": xT, "gu": gu_dev, "dn": dn_dev, "w": w_dev})
    return in_maps, slot_lists


def kernel(hidden_states, gate_weight, gate_up_proj, down_proj,
           merge_groups, dominant_experts):
    in_maps, slot_lists = prepare_in_maps(
        hidden_states, gate_weight, gate_up_proj, down_proj,
        merge_groups, dominant_experts)
    nc = _build()
    res = run_bass_kernel_spmd(nc, in_maps, core_ids=list(range(G)),
                               trace=False)
    out = np.zeros((T, H), dtype=np.float64)
    for c, r in enumerate(res.results):
        y = np.asarray(r["y"], dtype=np.float64)             # [NT, H]
        slots = slot_lists[c]
        # segments have unique tokens internally; add per segment
        for so, n in zip(TOFF, SEGT):
            sl = slots[so:so + n]
            valid = sl >= 0
            out[sl[valid]] += y[so:so + n][valid]
    return out.astype(np.float32).reshape(1, T, H)
